# revision 4
# baseline (speedup 1.0000x reference)
"""Trainium2 Bass kernel for nn_ConfidenceFilter (3-layer MLP with per-sample
early exit on softmax confidence).

Reference computation (B=8192, D=H=2048, C=1000):
    h1 = relu(x@W1+b1); p1 = h1@H1w+H1b; c1 = max softmax(p1) > 0.01
    h2 = relu(h1@W2+b2); p2 = h2@H2w+H2b; c2 = max softmax(p2) > 0.01
    h3 = relu(h2@W3+b3); p3 = h3@Fw+Fb
    out = where(c1, p1, where(c2, p2, p3))

Sharding: pure data parallel over 8 NeuronCores (1024 batch rows each), all
weights replicated; batch processed in two halves of 512 rows per core so
activations fit SBUF.

Layout: activations live transposed in SBUF (hT = [feature_part, batch]) so
backbone layers chain stationary=W-chunk / moving=hT; heads flip to
stationary=hT-chunk / moving=Hw-slice producing logits [batch_part, class],
making the confidence reduction a cheap free-dim reduce + ScalarE
exp-accumulate (max softmax prob > t  <=>  sum exp(p - max) < 1/t).

Precision: float32r operands are RNE-rounded to 11 mantissa bits but stream
at full PE rate (fp32 streams at half rate, as two half-speed passes). The c1
mask has samples 2.7e-5 (rel) from the threshold, so x@W1 and h1@H1w use a
3-pass split: with xhi=rne11(x), xr=x-xhi (both f32r) and likewise W, the sum
Whi@xhi + Whi@xr + Wr@xhi reproduces fp32-grade logits (~1e-6) at 3 cycles/row
instead of fp32's 4. The h1 residual is carried in bf16 and paired with a bf16
copy of H1w (walrus requires matmul operand dtypes to match when either is
fp32/f32r). The c2 mask margin is 4.6e-2 and p2/p3 only contribute output
values, so W2/H2w/W3/Fw run single-pass f32r (~3e-4).
"""

import numpy as np
import ml_dtypes
from contextlib import ExitStack

import concourse.bass as bass
import concourse.mybir as mybir
import concourse.tile as tile
from concourse import bacc
from concourse.bass_utils import run_bass_kernel_spmd

f32 = mybir.dt.float32
f32r = mybir.dt.float32r
bf16 = mybir.dt.bfloat16
AF = mybir.ActivationFunctionType
OP = mybir.AluOpType
AX = mybir.AxisListType

N_CORES = 8
THRESH_INV = 100.0  # 1/0.01: confident iff sum(exp(p - max)) < 100
USE_JOINS = True


def build(D=2048, H=2048, C=1000, BC=1024, HALF=512, split3=True):
    KC = D // 128          # k chunks for layer 1
    NC = H // 128          # hidden chunks (k chunks for layers 2/3, heads)
    MC = HALF // 128       # batch chunks of 128 within a half
    NHALF = BC // HALF
    CW = C // 2            # class window (<=512)
    assert C % 2 == 0 and CW <= 512

    nc = bacc.Bacc("TRN2", target_bir_lowering=False, debug=False,
                   num_devices=N_CORES)

    def din(name, shape, dt=f32):
        return nc.dram_tensor(name, shape, dt, kind="ExternalInput").ap()

    xT = din("xT", [D, BC])
    xTr = din("xTr", [D, BC])                      # x - rne11(x)
    W = [din("W1", [D, H]), din("W2", [H, H]), din("W3", [H, H])]
    W1r = din("W1r", [D, H])                       # W1 - rne11(W1)
    bvec = [din("b1", [H]), din("b2", [H]), din("b3", [H])]
    Hw = [din("H1w", [H, C]), din("H2w", [H, C]), din("Fw", [H, C])]
    H1wr = din("H1wr", [H, C])                     # H1w - rne11(H1w)
    H1wb = din("H1wb", [H, C], bf16)               # bf16(H1w)
    Hb = [din("H1b", [1, C]), din("H2b", [1, C]), din("Fb", [1, C])]
    out = nc.dram_tensor("out", [BC, C], f32, kind="ExternalOutput").ap()

    with tile.TileContext(nc) as tc, ExitStack() as ctx:
        pool = lambda name, bufs, **kw: ctx.enter_context(
            tc.tile_pool(name=name, bufs=bufs, **kw))

        # activation slots; tags chosen so lifetimes chain without overlap:
        #  s1: xhi(h) -> h2(h);  s2: xr(h) -> h3(h);  s3: h1hi;  s4: h1r bf16
        sb_s1 = pool("s1", 1)
        sb_s2 = pool("s2", 1)
        sb_s3 = pool("s3", 1)
        sb_s4 = pool("s4", 1)
        sb_acc = pool("acc", 1)       # blend state [128,MC,C] fp32
        sb_wst = pool("wst", 4)       # backbone weight col-blocks [128,*,128]
        sb_hwst = pool("hwst", 6)     # head weight slices [128,CW] f32r
        sb_hwb = pool("hwb", 3)       # head weight slices [128,CW] bf16
        sb_hbc = pool("hbc", 2)       # head bias broadcast [128,CW]
        sb_bias = pool("bias", 3)     # backbone bias [128,NC]
        sb_esc = pool("esc", 2)       # exp scratch (write-only sink)
        sb_stage = pool("stage", 3)   # final out staging [128,CW]
        sb_dt = pool("dt", 3)         # blend diff [128,CW]
        sb_tmpf = pool("tmpf", 2)     # relu full-precision staging [128,HALF]
        sb_stat = pool("stat", 32)    # [128,1] stats
        sb_mask = pool("mask", 24)    # c1/f2 masks [128,1]
        sb_k = pool("k", 6)           # constants / junk

        ps_main = pool("ps", 7, space="PSUM")   # shared backbone+head psum
        ps_j = pool("psj", 1, space="PSUM")     # join target

        # ---- preamble ----
        zjoin = sb_k.tile([1, 1], f32, tag="zjoin")
        nc.vector.memset(zjoin[:], 0.0)
        zjoinb = sb_k.tile([1, 1], bf16, tag="zjoinb")
        nc.vector.memset(zjoinb[:], 0.0)
        jps = ps_j.tile([1, 64], f32, tag="jps")
        nc.tensor.matmul(jps[0:1, 0:1], lhsT=zjoin[:], rhs=zjoin[:],
                         start=True, stop=True)
        nc.tensor.matmul(jps[0:1, 0:1], lhsT=zjoinb[:], rhs=zjoinb[:],
                         start=True, stop=True)

        def pe_join(ap, is_bf16=False):
            """Absorb one fresh sem wait on PE via a tiny matmul so real
            matmuls keep <=1 wait (walrus limit)."""
            if not USE_JOINS:
                return
            nfree = ap.free_size()
            z = zjoinb if is_bf16 else zjoin
            a = ap if is_bf16 else ap.bitcast(f32)
            nc.tensor.matmul(jps[0:1, 0:nfree], lhsT=z[:], rhs=a,
                             start=True, stop=True)

        awarm = sb_k.tile([1, 1], f32, tag="awarm")
        nc.scalar.activation(awarm[:], zjoin[:], AF.Exp)  # load ACT exp table

        ajunk = sb_k.tile([1, 1], f32, tag="ajunk")
        vjunk = sb_k.tile([1, 1], f32, tag="vjunk")

        btiles = []
        for li in range(3):
            bt = sb_bias.tile([128, NC], f32, tag="bias")
            nc.sync.dma_start(bt[:], bvec[li].rearrange("(n p) -> p n", p=128))
            nc.scalar.copy(ajunk[:], bt[0:1, 0:1])  # ACT join on the DMA
            btiles.append(bt)

        def load_T(dst, src_f32r, kchunks, msl):
            """DMA [D|H, HALF] slice into [128, kchunks, HALF] tile, one DMA
            per k-chunk so consumers can start after the first chunk."""
            for kc in range(kchunks):
                nc.sync.dma_start(dst[:, kc, :],
                                  src_f32r[kc * 128:(kc + 1) * 128, msl])

        def backbone(li, passes, rhs_chunks, out_writer):
            """One backbone layer: for each hidden chunk n accumulate over kc
            and passes into PSUM, then out_writer(n, ps). passes = list of
            (w_dram_f32r_view, rhs_tile)."""
            for n in range(NC):
                nsl = slice(n * 128, (n + 1) * 128)
                wblks = []
                for pi, (wd, _) in enumerate(passes):
                    wblk = sb_wst.tile([128, rhs_chunks, 128], f32r, tag="wst")
                    nc.sync.dma_start(
                        wblk[:], wd[:, nsl].rearrange("(kc p) m -> p kc m",
                                                      p=128))
                    pe_join(wblk[0:1, 0, 0:1])
                    wblks.append(wblk)
                ps = ps_main.tile([128, HALF], f32, tag="ps")
                npass = len(passes)
                for kc in range(rhs_chunks):
                    for pi, (_, rt) in enumerate(passes):
                        nc.tensor.matmul(
                            ps[:], lhsT=wblks[pi][:, kc, :], rhs=rt[:, kc, :],
                            start=(kc == 0 and pi == 0),
                            stop=(kc == rhs_chunks - 1 and pi == npass - 1))
                out_writer(n, ps)

        def head(hi, h, passes, c1_masks, f2_masks, acc_t):
            """Head hi over pass list [(stat_ap_fn, w_dram_view, dtype)].
            hi 0: write p1 into acc, compute c1. hi 1: blend p2 via c1,
            f2 = c1|c2. hi 2: blend p3 via f2, DMA final rows."""
            Msav, Ssav = {}, {}
            for nw in range(2):
                csl = slice(nw * CW, (nw + 1) * CW)
                hbc = sb_hbc.tile([128, CW], f32, tag="hbc")
                nc.sync.dma_start(hbc[:],
                                  Hb[hi][0:1, csl].broadcast_to((128, CW)))
                nc.vector.tensor_copy(vjunk[:], hbc[0:1, 0:1])  # DVE join
                phs = []
                npass = len(passes)
                for kc in range(NC):
                    hws = []
                    for (sfn, wd, dt) in passes:
                        if dt == bf16:
                            hw = sb_hwb.tile([128, CW], bf16, tag="hwb")
                        else:
                            hw = sb_hwst.tile([128, CW], f32r, tag="hwst")
                        nc.sync.dma_start(
                            hw[:], wd[kc * 128:(kc + 1) * 128, csl])
                        pe_join(hw[0:1, 0:1], is_bf16=(dt == bf16))
                        hws.append(hw)
                    for mc in range(MC):
                        if kc == 0:
                            ph_new = ps_main.tile([128, CW], f32, tag="ps")
                            phs.append(ph_new)
                        for pi, (sfn, wd, dt) in enumerate(passes):
                            nc.tensor.matmul(
                                phs[mc][:], lhsT=sfn(kc, mc), rhs=hws[pi][:],
                                start=(kc == 0 and pi == 0),
                                stop=(kc == NC - 1 and pi == npass - 1))
                for mc in range(MC):
                    ph = phs[mc]
                    nc.vector.tensor_tensor(ph[:], ph[:], hbc[:], op=OP.add)
                    if hi < 2:
                        M = sb_stat.tile([128, 1], f32, tag="stat")
                        nc.vector.tensor_reduce(M[:], ph[:], axis=AX.X,
                                                op=OP.max)
                        negm = sb_stat.tile([128, 1], f32, tag="stat")
                        nc.vector.tensor_scalar(negm[:], M[:], -1.0, None,
                                                op0=OP.mult)
                        esc = sb_esc.tile([128, CW], f32, tag="esc")
                        S = sb_stat.tile([128, 1], f32, tag="stat")
                        nc.scalar.activation(esc[:], ph[:], AF.Exp,
                                             bias=negm[:], accum_out=S[:])
                        Msav[(mc, nw)] = M
                        Ssav[(mc, nw)] = S
                    if hi == 0:
                        nc.scalar.copy(acc_t[:, mc, csl], ph[:])
                    else:
                        sel = c1_masks[mc] if hi == 1 else f2_masks[mc]
                        dt_t = sb_dt.tile([128, CW], f32, tag="dt")
                        nc.vector.tensor_tensor(dt_t[:], acc_t[:, mc, csl],
                                                ph[:], op=OP.subtract)
                        dst = acc_t[:, mc, csl]
                        if hi == 2:
                            stg = sb_stage.tile([128, CW], f32, tag="stage")
                            dst = stg[:]
                        nc.vector.scalar_tensor_tensor(
                            dst, in0=dt_t[:], scalar=sel[:], in1=ph[:],
                            op0=OP.mult, op1=OP.add)
                        if hi == 2:
                            r0 = h * HALF + mc * 128
                            nc.gpsimd.dma_start(out[r0:r0 + 128, csl], stg[:])
            if hi == 2:
                return
            # combine windows: s = s0*exp(M0-M) + s1*exp(M1-M), M=max(M0,M1)
            for mc in range(MC):
                M0, M1 = Msav[(mc, 0)], Msav[(mc, 1)]
                S0, S1 = Ssav[(mc, 0)], Ssav[(mc, 1)]
                M = sb_stat.tile([128, 1], f32, tag="stat")
                nc.vector.tensor_tensor(M[:], M0[:], M1[:], op=OP.max)
                s_tot = sb_stat.tile([128, 1], f32, tag="stat")
                first = True
                for Mi, Si in ((M0, S0), (M1, S1)):
                    dd = sb_stat.tile([128, 1], f32, tag="stat")
                    nc.vector.tensor_tensor(dd[:], Mi[:], M[:],
                                            op=OP.subtract)
                    ee = sb_stat.tile([128, 1], f32, tag="stat")
                    nc.scalar.activation(ee[:], dd[:], AF.Exp)
                    tt = sb_stat.tile([128, 1], f32, tag="stat")
                    nc.vector.tensor_tensor(tt[:], Si[:], ee[:], op=OP.mult)
                    if first:
                        nc.vector.tensor_copy(s_tot[:], tt[:])
                        first = False
                    else:
                        nc.vector.tensor_tensor(s_tot[:], s_tot[:], tt[:],
                                                op=OP.add)
                c = sb_mask.tile([128, 1], f32, tag=f"c{hi}")
                nc.vector.tensor_scalar(c[:], s_tot[:], THRESH_INV, None,
                                        op0=OP.is_lt)
                if hi == 0:
                    c1_masks[mc] = c
                else:
                    f2 = sb_mask.tile([128, 1], f32, tag="f2")
                    nc.vector.tensor_tensor(f2[:], c1_masks[mc][:], c[:],
                                            op=OP.max)
                    f2_masks[mc] = f2

        def stat_fn(t):
            return lambda kc, mc: t[:, kc, mc * 128:(mc + 1) * 128]

        for h in range(NHALF):
            msl = slice(h * HALF, (h + 1) * HALF)
            xhi = sb_s1.tile([128, KC, HALF], f32r, tag="s1")
            load_T(xhi, xT.bitcast(f32r), KC, msl)
            h1hi = sb_s3.tile([128, NC, HALF], f32r, tag="s3")
            h1rb = sb_s4.tile([128, NC, HALF], bf16, tag="s4")

            if split3:
                xr = sb_s2.tile([128, KC, HALF], f32r, tag="s2")
                load_T(xr, xTr.bitcast(f32r), KC, msl)
                l1_passes = [(W[0].bitcast(f32r), xhi),
                             (W[0].bitcast(f32r), xr),
                             (W1r.bitcast(f32r), xhi)]
            else:
                l1_passes = [(W[0].bitcast(f32r), xhi)]

            def l1_writer(n, ps):
                tmp = sb_tmpf.tile([128, HALF], f32, tag="tmpf")
                nc.scalar.activation(tmp[:], ps[:], AF.Relu,
                                     bias=btiles[0][:, n:n + 1])
                nc.scalar.copy(h1hi[:, n, :], tmp[:])
                nc.vector.tensor_tensor(h1rb[:, n, :], tmp[:],
                                        h1hi[:, n, :].bitcast(f32),
                                        op=OP.subtract)
            backbone(0, l1_passes, KC, l1_writer)

            acc_t = sb_acc.tile([128, MC, C], f32, tag="acc")
            c1_masks, f2_masks = {}, {}
            pe_join(h1hi[0:1, :, 0:1])
            pe_join(h1rb[0:1, :, 0:1], is_bf16=True)
            if split3:
                h1_passes = [(stat_fn(h1hi), Hw[0].bitcast(f32r), f32r),
                             (stat_fn(h1hi), H1wr.bitcast(f32r), f32r),
                             (stat_fn(h1rb), H1wb, bf16)]
            else:
                h1_passes = [(stat_fn(h1hi), Hw[0].bitcast(f32r), f32r)]
            head(0, h, h1_passes, c1_masks, f2_masks, acc_t)

            h2_t = sb_s1.tile([128, NC, HALF], f32r, tag="s1")
            pe_join(h1hi[0:1, :, 0:1])

            def l2_writer(n, ps):
                nc.scalar.activation(h2_t[:, n, :], ps[:], AF.Relu,
                                     bias=btiles[1][:, n:n + 1])
            backbone(1, [(W[1].bitcast(f32r), h1hi)], NC, l2_writer)

            pe_join(h2_t[0:1, :, 0:1])
            head(1, h, [(stat_fn(h2_t), Hw[1].bitcast(f32r), f32r)],
                 c1_masks, f2_masks, acc_t)

            h3_t = sb_s2.tile([128, NC, HALF], f32r, tag="s2")

            def l3_writer(n, ps):
                nc.scalar.activation(h3_t[:, n, :], ps[:], AF.Relu,
                                     bias=btiles[2][:, n:n + 1])
            backbone(2, [(W[2].bitcast(f32r), h2_t)], NC, l3_writer)

            pe_join(h3_t[0:1, :, 0:1])
            head(2, h, [(stat_fn(h3_t), Hw[2].bitcast(f32r), f32r)],
                 c1_masks, f2_masks, acc_t)

    nc.compile()
    return nc


def _rne11(a):
    bits = np.ascontiguousarray(a, dtype=np.float32).view(np.uint32)
    half = np.uint32(1 << 11)
    lsb = (bits >> np.uint32(12)) & np.uint32(1)
    r = (bits + (half - np.uint32(1)) + lsb) & np.uint32(0xFFFFF000)
    return r.view(np.float32)


_cached = {}


def _get_nc():
    if "nc" not in _cached:
        _cached["nc"] = build()
    return _cached["nc"]


def kernel(x, W1, b1, W2, b2, W3, b3, H1w, H1b, H2w, H2b, Fw, Fb,
           _trace=False):
    x = np.ascontiguousarray(np.asarray(x, dtype=np.float32))
    B = x.shape[0]
    BC = B // N_CORES
    C = np.asarray(H1w).shape[1]
    f = lambda a: np.ascontiguousarray(np.asarray(a, dtype=np.float32))
    W1 = f(W1)
    H1w = f(H1w)
    common = {
        "W1": W1, "W2": f(W2), "W3": f(W3),
        "W1r": W1 - _rne11(W1),
        "b1": f(b1), "b2": f(b2), "b3": f(b3),
        "H1w": H1w, "H2w": f(H2w), "Fw": f(Fw),
        "H1wr": H1w - _rne11(H1w),
        "H1wb": H1w.astype(ml_dtypes.bfloat16),
        "H1b": f(H1b).reshape(1, C), "H2b": f(H2b).reshape(1, C),
        "Fb": f(Fb).reshape(1, C),
    }
    in_maps = []
    for c in range(N_CORES):
        xTc = np.ascontiguousarray(x[c * BC:(c + 1) * BC].T)
        in_maps.append({"xT": xTc, "xTr": xTc - _rne11(xTc), **common})
    nc = _get_nc()
    res = run_bass_kernel_spmd(nc, in_maps, core_ids=list(range(N_CORES)),
                               trace=_trace)
    kernel._last_exec_time_ns = res.exec_time_ns
    return np.concatenate([res.results[c]["out"] for c in range(N_CORES)],
                          axis=0)


# revision 10
# speedup vs baseline: 1.0317x; 1.0317x over previous
"""Trainium2 Bass kernel for nn_ConfidenceFilter (3-layer MLP with per-sample
early exit on softmax confidence).

Reference computation (B=8192, D=H=2048, C=1000):
    h1 = relu(x@W1+b1); p1 = h1@H1w+H1b; c1 = max softmax(p1) > 0.01
    h2 = relu(h1@W2+b2); p2 = h2@H2w+H2b; c2 = max softmax(p2) > 0.01
    h3 = relu(h2@W3+b3); p3 = h3@Fw+Fb
    out = where(c1, p1, where(c2, p2, p3))

Sharding: pure data parallel over 8 NeuronCores (1024 batch rows each), all
weights replicated; batch processed in two halves of 512 rows per core so
activations fit SBUF.

Layout: activations live transposed in SBUF (hT = [feature_part, batch]) so
backbone layers chain stationary=W-chunk / moving=hT; heads flip to
stationary=hT-chunk / moving=Hw-slice producing logits [batch_part, class],
making the confidence reduction a cheap free-dim reduce + ScalarE
exp-accumulate (max softmax prob > t  <=>  sum exp(p - max) < 1/t).

Precision: float32r operands are RNE-rounded to 11 mantissa bits but stream
at full PE rate (fp32 streams at half rate as two half-speed passes), and
11x11-bit products accumulate exactly into fp32 PSUM. The c1 mask has samples
2.7e-5 (rel) from the threshold, so x@W1 and h1@H1w are computed with 3-pass
operand splits accurate to ~1e-6:
    x@W1   ~= Whi@xhi + Whi@xr + Wrb@xhib      (xhi=rne11(x), xr=x-xhi f32r;
                                                Wrb=bf16(W1-rne11(W1)),
                                                xhib=bf16(xhi) on-chip)
    h1@H1w ~= Hwhi@h1hi + Hwr@h1hi + Hwhi@h1r  (h1r = relu - h1hi, DVE f32r)
where each pass reuses an already-streamed weight block where possible to
keep HBM traffic down (the kernel is near the DMA roofline). The c2 mask
margin is 4.6e-2 and p2/p3 only contribute output values, so W2/H2w/W3/Fw
run single-pass f32r (~3e-4 relative output error).
"""

import numpy as np
import ml_dtypes
from contextlib import ExitStack

import concourse.bass as bass
import concourse.mybir as mybir
import concourse.tile as tile
from concourse import bacc
from concourse.bass_utils import run_bass_kernel_spmd

f32 = mybir.dt.float32
f32r = mybir.dt.float32r
bf16 = mybir.dt.bfloat16
AF = mybir.ActivationFunctionType
OP = mybir.AluOpType
AX = mybir.AxisListType

N_CORES = 8
THRESH_INV = 100.0  # 1/0.01: confident iff sum(exp(p - max)) < 100
USE_JOINS = True


def build(D=2048, H=2048, C=1000, BC=1024, HALF=512, split3=True):
    KC = D // 128          # k chunks for layer 1
    NC = H // 128          # hidden chunks (k chunks for layers 2/3, heads)
    MC = HALF // 128       # batch chunks of 128 within a half
    NHALF = BC // HALF
    CW = C // 2            # class window (<=512)
    assert C % 2 == 0 and CW <= 512

    nc = bacc.Bacc("TRN2", target_bir_lowering=False, debug=False,
                   num_devices=N_CORES)

    def din(name, shape, dt=f32):
        return nc.dram_tensor(name, shape, dt, kind="ExternalInput").ap()

    xT = din("xT", [D, BC])
    xTr = din("xTr", [D, BC])                      # x - rne11(x)
    W = [din("W1", [D, H]), din("W2", [H, H]), din("W3", [H, H])]
    W1rb = din("W1rb", [D, H], bf16)               # bf16(W1 - rne11(W1))
    bvec = [din("b1", [H]), din("b2", [H]), din("b3", [H])]
    Hw = [din("H1w", [H, C]), din("H2w", [H, C]), din("Fw", [H, C])]
    H1wr = din("H1wr", [H, C])                     # H1w - rne11(H1w)
    Hb = [din("H1b", [1, C]), din("H2b", [1, C]), din("Fb", [1, C])]
    out = nc.dram_tensor("out", [BC, C], f32, kind="ExternalOutput").ap()

    with tile.TileContext(nc) as tc, ExitStack() as ctx:
        pool = lambda name, bufs, **kw: ctx.enter_context(
            tc.tile_pool(name=name, bufs=bufs, **kw))

        # activation slots; tags chosen so lifetimes chain without overlap:
        #  s1: xhi(h) -> h2(h);  s2: xr(h) -> h3(h);  s3: h1hi;  s4: h1r
        sb_s1 = pool("s1", 1)
        sb_s2 = pool("s2", 1)
        sb_s3 = pool("s3", 1)
        sb_s4 = pool("s4", 1)
        sb_xb = pool("xb", 1)         # bf16(xhi) for the L1 bf16 pass
        sb_acc = pool("acc", 1)       # blend state [128,MC,C] fp32
        sb_wst = pool("wst", 2)       # backbone f32r weight blocks [128,*,128]
        sb_wbt = pool("wbt", 2)       # backbone bf16 weight blocks
        sb_hwst = pool("hwst", 4)     # head weight slices [128,CW] f32r
        sb_hbc = pool("hbc", 2)       # head bias broadcast [128,CW]
        sb_bias = pool("bias", 3)     # backbone bias [128,NC]
        sb_esc = pool("esc", 1)       # exp scratch bf16 (write-only sink)
        sb_stage = pool("stage", 1)   # final out staging [128,CW]
        sb_tmpf = pool("tmpf", 1)     # relu full-precision staging [128,HALF]
        sb_stat = pool("stat", 28)    # [128,1] stats
        sb_mask = pool("mask", 8)    # c1/f2 masks [128,1]
        sb_k = pool("k", 6)           # constants / junk

        ps_main = pool("ps", 7, space="PSUM")   # shared backbone+head psum
        ps_j = pool("psj", 1, space="PSUM")     # join target

        # ---- preamble ----
        zjoin = sb_k.tile([1, 1], f32, tag="zjoin")
        nc.vector.memset(zjoin[:], 0.0)
        zjoinb = sb_k.tile([1, 1], bf16, tag="zjoinb")
        nc.vector.memset(zjoinb[:], 0.0)
        jps = ps_j.tile([1, 64], f32, tag="jps")
        nc.tensor.matmul(jps[0:1, 0:1], lhsT=zjoin[:], rhs=zjoin[:],
                         start=True, stop=True)
        nc.tensor.matmul(jps[0:1, 0:1], lhsT=zjoinb[:], rhs=zjoinb[:],
                         start=True, stop=True)

        def pe_join(ap, is_bf16=False):
            """Absorb one fresh sem wait on PE via a tiny matmul so real
            matmuls keep <=1 wait (walrus limit)."""
            if not USE_JOINS:
                return
            nfree = ap.free_size()
            z = zjoinb if is_bf16 else zjoin
            a = ap if is_bf16 else ap.bitcast(f32)
            nc.tensor.matmul(jps[0:1, 0:nfree], lhsT=z[:], rhs=a,
                             start=True, stop=True)

        awarm = sb_k.tile([1, 1], f32, tag="awarm")
        nc.scalar.activation(awarm[:], zjoin[:], AF.Exp)  # load ACT exp table

        ajunk = sb_k.tile([1, 1], f32, tag="ajunk")
        vjunk = sb_k.tile([1, 1], f32, tag="vjunk")

        btiles = []
        for li in range(3):
            bt = sb_bias.tile([128, NC], f32, tag="bias")
            nc.sync.dma_start(bt[:], bvec[li].rearrange("(n p) -> p n", p=128))
            nc.scalar.copy(ajunk[:], bt[0:1, 0:1])  # ACT join on the DMA
            btiles.append(bt)

        def backbone(streams, mms, rhs_chunks, out_writer):
            """streams: [(w_dram_view, dtype)] weight column-block streams.
            mms: [(stream_idx, rhs_tile)] matmul passes per k-chunk.
            For each hidden chunk n: DMA the blocks, accumulate all passes
            over kc into PSUM, then out_writer(n, ps)."""
            for n in range(NC):
                nsl = slice(n * 128, (n + 1) * 128)
                wblks = []
                for (wd, dt) in streams:
                    pl = sb_wbt if dt == bf16 else sb_wst
                    wblk = pl.tile([128, rhs_chunks, 128], dt,
                                   tag="wbt" if dt == bf16 else "wst")
                    nc.sync.dma_start(
                        wblk[:], wd[:, nsl].rearrange("(kc p) m -> p kc m",
                                                      p=128))
                    pe_join(wblk[0:1, 0, 0:1], is_bf16=(dt == bf16))
                    wblks.append(wblk)
                ps = ps_main.tile([128, HALF], f32, tag="ps")
                npass = len(mms)
                for kc in range(rhs_chunks):
                    for pi, (si, rt) in enumerate(mms):
                        nc.tensor.matmul(
                            ps[:], lhsT=wblks[si][:, kc, :], rhs=rt[:, kc, :],
                            start=(kc == 0 and pi == 0),
                            stop=(kc == rhs_chunks - 1 and pi == npass - 1))
                out_writer(n, ps)

        def head(hi, h, streams, mms, c1_masks, f2_masks, acc_t):
            """Head hi. streams: [(w_dram_view, dtype)] moving slices per kc;
            mms: [(stream_idx, stat_ap_fn)] passes.
            hi 0: write p1 into acc + compute c1; hi 1: blend p2 via c1 and
            f2 = c1|c2; hi 2: blend p3 via f2 and DMA final rows."""
            Msav, Ssav = {}, {}
            for nw in range(2):
                csl = slice(nw * CW, (nw + 1) * CW)
                hbc = sb_hbc.tile([128, CW], f32, tag="hbc")
                nc.sync.dma_start(hbc[:],
                                  Hb[hi][0:1, csl].broadcast_to((128, CW)))
                nc.vector.tensor_copy(vjunk[:], hbc[0:1, 0:1])  # DVE join
                phs = []
                npass = len(mms)
                for kc in range(NC):
                    hws = []
                    for (wd, dt) in streams:
                        hw = sb_hwst.tile([128, CW], dt, tag="hwst")
                        nc.sync.dma_start(
                            hw[:], wd[kc * 128:(kc + 1) * 128, csl])
                        pe_join(hw[0:1, 0:1], is_bf16=(dt == bf16))
                        hws.append(hw)
                    for mc in range(MC):
                        if kc == 0:
                            ph_new = ps_main.tile([128, CW], f32, tag="ps")
                            phs.append(ph_new)
                        for pi, (si, sfn) in enumerate(mms):
                            nc.tensor.matmul(
                                phs[mc][:], lhsT=sfn(kc, mc), rhs=hws[si][:],
                                start=(kc == 0 and pi == 0),
                                stop=(kc == NC - 1 and pi == npass - 1))
                for mc in range(MC):
                    ph = phs[mc]
                    nc.vector.tensor_tensor(ph[:], ph[:], hbc[:], op=OP.add)
                    if hi < 2:
                        M = sb_stat.tile([128, 1], f32, tag="stat")
                        nc.vector.tensor_reduce(M[:], ph[:], axis=AX.X,
                                                op=OP.max)
                        negm = sb_stat.tile([128, 1], f32, tag="stat")
                        nc.vector.tensor_scalar(negm[:], M[:], -1.0, None,
                                                op0=OP.mult)
                        esc = sb_esc.tile([128, CW], bf16, tag="esc")
                        S = sb_stat.tile([128, 1], f32, tag="stat")
                        nc.scalar.activation(esc[:], ph[:], AF.Exp,
                                             bias=negm[:], accum_out=S[:])
                        Msav[(mc, nw)] = M
                        Ssav[(mc, nw)] = S
                    if hi == 0:
                        nc.scalar.copy(acc_t[:, mc, csl], ph[:])
                    else:
                        # out = sel*acc + (1-sel)*ph, via in-place acc scale
                        sel, nsel = (c1_masks[mc] if hi == 1
                                     else f2_masks[mc])
                        nc.vector.tensor_scalar(acc_t[:, mc, csl],
                                                acc_t[:, mc, csl], sel[:],
                                                None, op0=OP.mult)
                        dst = acc_t[:, mc, csl]
                        if hi == 2:
                            stg = sb_stage.tile([128, CW], f32, tag="stage")
                            dst = stg[:]
                        nc.vector.scalar_tensor_tensor(
                            dst, in0=ph[:], scalar=nsel[:],
                            in1=acc_t[:, mc, csl], op0=OP.mult, op1=OP.add)
                        if hi == 2:
                            r0 = h * HALF + mc * 128
                            nc.gpsimd.dma_start(out[r0:r0 + 128, csl], stg[:])
            if hi == 2:
                return
            # combine windows: s = s0*exp(M0-M) + s1*exp(M1-M), M=max(M0,M1)
            for mc in range(MC):
                M0, M1 = Msav[(mc, 0)], Msav[(mc, 1)]
                S0, S1 = Ssav[(mc, 0)], Ssav[(mc, 1)]
                M = sb_stat.tile([128, 1], f32, tag="stat")
                nc.vector.tensor_tensor(M[:], M0[:], M1[:], op=OP.max)
                s_tot = sb_stat.tile([128, 1], f32, tag="stat")
                first = True
                for Mi, Si in ((M0, S0), (M1, S1)):
                    dd = sb_stat.tile([128, 1], f32, tag="stat")
                    nc.vector.tensor_tensor(dd[:], Mi[:], M[:],
                                            op=OP.subtract)
                    ee = sb_stat.tile([128, 1], f32, tag="stat")
                    nc.scalar.activation(ee[:], dd[:], AF.Exp)
                    tt = sb_stat.tile([128, 1], f32, tag="stat")
                    nc.vector.tensor_tensor(tt[:], Si[:], ee[:], op=OP.mult)
                    if first:
                        nc.vector.tensor_copy(s_tot[:], tt[:])
                        first = False
                    else:
                        nc.vector.tensor_tensor(s_tot[:], s_tot[:], tt[:],
                                                op=OP.add)
                c = sb_mask.tile([128, 1], f32, tag=f"c{hi}")
                nc.vector.tensor_scalar(c[:], s_tot[:], THRESH_INV, None,
                                        op0=OP.is_lt)
                ncm = sb_mask.tile([128, 1], f32, tag=f"nc{hi}")
                nc.vector.tensor_scalar(ncm[:], s_tot[:], THRESH_INV, None,
                                        op0=OP.is_ge)
                if hi == 0:
                    c1_masks[mc] = (c, ncm)
                else:
                    f2 = sb_mask.tile([128, 1], f32, tag="f2")
                    nc.vector.tensor_tensor(f2[:], c1_masks[mc][0][:], c[:],
                                            op=OP.max)
                    nf2 = sb_mask.tile([128, 1], f32, tag="nf2")
                    nc.vector.tensor_tensor(nf2[:], c1_masks[mc][1][:],
                                            ncm[:], op=OP.min)
                    f2_masks[mc] = (f2, nf2)

        def stat_fn(t):
            return lambda kc, mc: t[:, kc, mc * 128:(mc + 1) * 128]

        for h in range(NHALF):
            msl = slice(h * HALF, (h + 1) * HALF)
            xhi = sb_s1.tile([128, KC, HALF], f32r, tag="s1")
            if split3:
                xhib = sb_xb.tile([128, KC, HALF], bf16, tag="xb")
                xr = sb_s2.tile([128, KC, HALF], f32r, tag="s2")
            for kc in range(KC):
                ksl = slice(kc * 128, (kc + 1) * 128)
                nc.sync.dma_start(xhi[:, kc, :], xT.bitcast(f32r)[ksl, msl])
                if split3:
                    nc.vector.tensor_copy(xhib[:, kc, :], xhi[:, kc, :])
                    nc.sync.dma_start(xr[:, kc, :],
                                      xTr.bitcast(f32r)[ksl, msl])
            h1hi = sb_s3.tile([128, NC, HALF], f32r, tag="s3")
            h1r = sb_s4.tile([128, NC, HALF], f32r, tag="s4")

            if split3:
                l1_streams = [(W[0].bitcast(f32r), f32r), (W1rb, bf16)]
                l1_mms = [(0, xhi), (0, xr), (1, xhib)]
            else:
                l1_streams = [(W[0].bitcast(f32r), f32r)]
                l1_mms = [(0, xhi)]

            def l1_writer(n, ps):
                tmp = sb_tmpf.tile([128, HALF], f32, tag="tmpf")
                nc.scalar.activation(tmp[:], ps[:], AF.Relu,
                                     bias=btiles[0][:, n:n + 1])
                nc.scalar.copy(h1hi[:, n, :], tmp[:])
                nc.vector.tensor_tensor(h1r[:, n, :], tmp[:],
                                        h1hi[:, n, :].bitcast(f32),
                                        op=OP.subtract)
            backbone(l1_streams, l1_mms, KC, l1_writer)

            acc_t = sb_acc.tile([128, MC, C], f32, tag="acc")
            c1_masks, f2_masks = {}, {}
            pe_join(h1hi[0:1, :, 0:1])
            pe_join(h1r[0:1, :, 0:1])
            if split3:
                h1_streams = [(Hw[0].bitcast(f32r), f32r),
                              (H1wr.bitcast(f32r), f32r)]
                h1_mms = [(0, stat_fn(h1hi)), (1, stat_fn(h1hi)),
                          (0, stat_fn(h1r))]
            else:
                h1_streams = [(Hw[0].bitcast(f32r), f32r)]
                h1_mms = [(0, stat_fn(h1hi))]
            head(0, h, h1_streams, h1_mms, c1_masks, f2_masks, acc_t)

            h2_t = sb_s1.tile([128, NC, HALF], f32r, tag="s1")
            pe_join(h1hi[0:1, :, 0:1])

            def l2_writer(n, ps):
                nc.scalar.activation(h2_t[:, n, :], ps[:], AF.Relu,
                                     bias=btiles[1][:, n:n + 1])
            backbone([(W[1].bitcast(f32r), f32r)], [(0, h1hi)], NC, l2_writer)

            pe_join(h2_t[0:1, :, 0:1])
            head(1, h, [(Hw[1].bitcast(f32r), f32r)], [(0, stat_fn(h2_t))],
                 c1_masks, f2_masks, acc_t)

            h3_t = sb_s2.tile([128, NC, HALF], f32r, tag="s2")

            def l3_writer(n, ps):
                nc.scalar.activation(h3_t[:, n, :], ps[:], AF.Relu,
                                     bias=btiles[2][:, n:n + 1])
            backbone([(W[2].bitcast(f32r), f32r)], [(0, h2_t)], NC, l3_writer)

            pe_join(h3_t[0:1, :, 0:1])
            head(2, h, [(Hw[2].bitcast(f32r), f32r)], [(0, stat_fn(h3_t))],
                 c1_masks, f2_masks, acc_t)

    nc.compile()
    return nc


def _rne11(a):
    bits = np.ascontiguousarray(a, dtype=np.float32).view(np.uint32)
    half = np.uint32(1 << 11)
    lsb = (bits >> np.uint32(12)) & np.uint32(1)
    r = (bits + (half - np.uint32(1)) + lsb) & np.uint32(0xFFFFF000)
    return r.view(np.float32)


_cached = {}


def _get_nc():
    if "nc" not in _cached:
        _cached["nc"] = build()
    return _cached["nc"]


def kernel(x, W1, b1, W2, b2, W3, b3, H1w, H1b, H2w, H2b, Fw, Fb,
           _trace=False):
    x = np.ascontiguousarray(np.asarray(x, dtype=np.float32))
    B = x.shape[0]
    BC = B // N_CORES
    C = np.asarray(H1w).shape[1]
    f = lambda a: np.ascontiguousarray(np.asarray(a, dtype=np.float32))
    W1 = f(W1)
    H1w = f(H1w)
    common = {
        "W1": W1, "W2": f(W2), "W3": f(W3),
        "W1rb": (W1 - _rne11(W1)).astype(ml_dtypes.bfloat16),
        "b1": f(b1), "b2": f(b2), "b3": f(b3),
        "H1w": H1w, "H2w": f(H2w), "Fw": f(Fw),
        "H1wr": H1w - _rne11(H1w),
        "H1b": f(H1b).reshape(1, C), "H2b": f(H2b).reshape(1, C),
        "Fb": f(Fb).reshape(1, C),
    }
    in_maps = []
    for c in range(N_CORES):
        xTc = np.ascontiguousarray(x[c * BC:(c + 1) * BC].T)
        in_maps.append({"xT": xTc, "xTr": xTc - _rne11(xTc), **common})
    nc = _get_nc()
    res = run_bass_kernel_spmd(nc, in_maps, core_ids=list(range(N_CORES)),
                               trace=_trace)
    kernel._last_exec_time_ns = res.exec_time_ns
    return np.concatenate([res.results[c]["out"] for c in range(N_CORES)],
                          axis=0)
